# revision 16
# baseline (speedup 1.0000x reference)
"""Trainium2 Bass kernel for a 2-layer GCN (GCNConv -> ReLU -> GCNConv -> log_softmax).

Strategy (8 NeuronCores, SPMD, one NEFF):
  * Nodes range-sharded by destination; each core owns N/8 dst nodes and all
    edges pointing at them.  out = D^-1/2 (A+I) D^-1/2 h factorizes: tables
    hold g = dinv * h; epilogue does (agg + g_own) * dinv + b.
  * Layer-2 applies W2 EARLY:  A(H1 W2) = (A H1) W2, so the layer-2 table is
    g1 = dinv*(relu(l1) @ W2) [N, 32] and layer-2 agg matmuls are 32 wide.
  * Tables are bf16, 128 B per node (64 feats; layer2: 32 feats + 32 zero
    pad).  dma_gather rows must be 256 B, so gathers fetch NODE PAIRS:
    idx = src//2 and the one-hot S matmul reads the parity half
    (rhs columns parity*64 + [0..W)).  Runs are parity-pure (host-sorted),
    and slots within each run are sorted by source row (HBM locality).
  * 2 bank windows of 4 cores (25088 packed rows < int16 range); runs per
    dst block = 2 banks x 2 parities = 4.  The SAME gidx + dstrel data
    serve BOTH layers (identical edge structure, only the table differs).
  * gidx lives resident in SBUF (one load); no per-gather index DMAs.
  * Own-slice tables (self-loop terms) live resident in SBUF (own0/own1).
  * One-hot S matrices are built on DVE in fp8e4 and feed the PE as the
    fp8 stationary against bf16 moving messages (mixed-dtype matmul).
  * Per-layer AllGather of the 1.6 MB bf16 slice (~60 us each).

Measured on HW: 1.349 ms (baseline this replaced: 1.92 ms).  Bottleneck is
the gpsimd SWDGE descriptor generation for the per-edge gathers (~2.5 us
per 1024-idx dma_gather, ~82% engine busy); the 1024-desc carveout is a
hard runtime limit (bigger instructions hang await_space), so per-edge
descgen cost is the floor of this design.
"""

import os
import sys
import numpy as np

P = 128
FEAT = 64
OUTC = 32
SG_BLOCKS = 7
GMAX = 1024      # max idxs per dma_gather instruction. HARD LIMIT: the
                 # SWDGE descriptor carveout is fixed at 1024 descs by the
                 # runtime; 1984/2048-idx instructions hang await_space
                 # (tested on reset devices, dynamic_dma_scratch_size has
                 # no effect).
SB = 32          # S-build batch (dcols per is_equal op)
S_FP8 = True     # one-hot S matrices in fp8e4 (stationary side of the
                 # segment-sum matmuls; moving side stays bf16)


class _PhaseDone(Exception):
    pass


class Schedule:
    pass


# --------------------------------------------------------------------------
# Host-side schedule construction
# --------------------------------------------------------------------------

def build_schedule(src, dst, n_nodes, n_cores):
    """Edges grouped per (dst core, dst 128-block, src bank-window, src
    parity); run lengths 16-quantized at the max over cores (SPMD shared
    schedule).  Gather idx = packed node-pair row within the bank window."""
    Q = 1    # run offsets need no alignment (gidx wrap only needs L%128==0)
    nslice = n_nodes // n_cores
    nblk = (nslice + P - 1) // P
    nsp = nblk * P                    # padded slice rows (node granularity)
    prows = nsp // 2                  # packed pair-rows per core
    nbank = 2
    halfrows = prows // 2             # packed rows per half-slice
    bankrows = n_cores * halfrows     # bank window = all cores' half-slices
    assert bankrows <= 32767

    src_a = src.astype(np.int64)
    dst_a = dst.astype(np.int64)
    core = dst_a // nslice
    block = (dst_a % nslice) // P
    sc = src_a // nslice
    sl = src_a % nslice
    bank = sl // (2 * halfrows)       # which half of the owner's slice
    par = sl % 2
    pidx = sc * halfrows + (sl // 2) % halfrows

    key = ((core * nblk + block) * nbank + bank) * 2 + par
    counts = np.bincount(key, minlength=n_cores * nblk * nbank * 2).reshape(
        n_cores, nblk, nbank, 2)
    R = (np.ceil(counts.max(axis=0) / Q) * Q).astype(np.int64)
    for b in range(nblk):
        if R[b].sum() == 0:
            R[b, 0, 0] = Q

    sgs = [list(range(i, min(i + SG_BLOCKS, nblk)))
           for i in range(0, nblk, SG_BLOCKS)]

    sch = Schedule()
    sch.n_nodes, sch.n_cores, sch.nslice = n_nodes, n_cores, nslice
    sch.nblk, sch.nsp, sch.prows = nblk, nsp, prows
    sch.nbank, sch.bankrows, sch.halfrows = nbank, bankrows, halfrows
    sch.sgs = sgs
    sch.R = R

    # per (sg, bank): run offsets and padded gather length
    sch.run_off = []     # [s_i][(blk, b, par)] -> slot offset
    sch.gather_L = []    # [s_i][b] -> padded length (128-multiple)
    for s_i, blks in enumerate(sgs):
        offs = {}
        Ls = []
        for b_i in range(nbank):
            o = 0
            for blk in blks:
                for pr in range(2):
                    offs[(blk, b_i, pr)] = o
                    o += int(R[blk, b_i, pr])
            o = ((o + P - 1) // P) * P
            Ls.append(o)
        sch.run_off.append(offs)
        sch.gather_L.append(Ls)
    sch.maxL = max(max(Ls) for Ls in sch.gather_L)

    # matmul sequence: per sg, block-major; entries are run-tile overlaps.
    # Entry: (b_i, par, tc, dcol, blk, start, stop)
    sch.mmseq = []
    ncols = 0
    dcol_map = []
    for s_i, blks in enumerate(sgs):
        seq = []
        for blk in blks:
            lst = []
            for b_i in range(nbank):
                for pr in range(2):
                    r0 = sch.run_off[s_i][(blk, b_i, pr)]
                    r1 = r0 + int(R[blk, b_i, pr])
                    if r1 == r0:
                        continue
                    t0, t1 = r0 // P, (r1 - 1) // P
                    for tc in range(t0, t1 + 1):
                        lst.append((b_i, pr, tc))
            for i, (b_i, pr, tc) in enumerate(lst):
                seq.append([b_i, pr, tc, ncols, blk, i == 0,
                            i == len(lst) - 1])
                dcol_map.append((s_i, b_i, pr, tc, blk))
                ncols += 1
        sch.mmseq.append(seq)
    sch.ncols = ncols
    sch.dcol_map = dcol_map

    # gidx column layout
    off = 0
    gidx_off = {}
    for s_i in range(len(sgs)):
        for b_i in range(nbank):
            gidx_off[(s_i, b_i)] = off
            off += sch.gather_L[s_i][b_i] // 16
    sch.gidx_cols = off
    sch.gidx_off = gidx_off

    # ---------------- per-core arrays ----------------
    # pidx as the fastest key: slots within each run come sorted by source
    # row, so gather descriptor streams walk increasing HBM addresses.
    order = np.lexsort((pidx, par, bank, block, core))
    p_o = pidx[order]
    d_o = dst_a[order]
    grp_key = key[order]
    uniq, starts = np.unique(grp_key, return_index=True)
    grp_start = {int(k): int(v) for k, v in zip(uniq, starts)}
    grp_count = {int(k): int(v) for k, v in
                 zip(uniq, np.diff(np.append(starts, len(grp_key))))}

    sch.core_gidx = []
    sch.core_dstrel = []
    for c in range(n_cores):
        gidx = np.zeros((16, max(sch.gidx_cols, 16)), dtype=np.int16)
        dstrel = np.full((P, sch.ncols), -1.0, dtype=np.float32)
        slot_dst = {}
        for s_i, blks in enumerate(sgs):
            for b_i in range(nbank):
                L = sch.gather_L[s_i][b_i]
                idx_lin = np.zeros(L, dtype=np.int16)
                dst_lin = np.full(L, -1, dtype=np.int64)
                for blk in blks:
                    for pr in range(2):
                        k = int((((c * nblk + blk) * nbank + b_i) * 2 + pr))
                        cnt = grp_count.get(k, 0)
                        if not cnt:
                            continue
                        st = grp_start[k]
                        o = sch.run_off[s_i][(blk, b_i, pr)]
                        idx_lin[o:o + cnt] = p_o[st:st + cnt].astype(np.int16)
                        dst_lin[o:o + cnt] = d_o[st:st + cnt]
                go = gidx_off[(s_i, b_i)]
                gidx[:, go:go + L // 16] = idx_lin.reshape(L // 16, 16).T
                slot_dst[(s_i, b_i)] = dst_lin
        for dcol, (s_i, b_i, pr, tc, blk) in enumerate(dcol_map):
            dl = slot_dst.get((s_i, b_i))
            if dl is None:
                continue
            r0 = sch.run_off[s_i][(blk, b_i, pr)]
            r1 = r0 + int(sch.R[blk, b_i, pr])
            lo = max(r0, tc * P)
            hi = min(r1, (tc + 1) * P)
            if hi <= lo:
                continue
            seg = dl[lo:hi]
            base = c * nslice + blk * P
            vals = seg - base
            vals = np.where((seg >= 0) & (vals >= 0) & (vals < P),
                            vals, -1).astype(np.float32)
            dstrel[lo - tc * P:hi - tc * P, dcol] = vals
        sch.core_gidx.append(np.tile(gidx, (8, 1)))
        sch.core_dstrel.append(dstrel)
    return sch


def numpy_check_schedule(sch, src, dst, n_nodes):
    """Emulate the device aggregation (no self loops) in numpy, both widths."""
    rng = np.random.default_rng(0)
    n_cores = sch.n_cores
    g = rng.standard_normal((n_nodes, FEAT)).astype(np.float32)
    ref = np.zeros((n_nodes, FEAT), np.float32)
    np.add.at(ref, dst, g[src])
    # build the packed half-major table [2 * bankrows, 128]
    T = np.zeros((2 * sch.bankrows, 2 * FEAT), np.float32)
    for c in range(n_cores):
        rows = g[c * sch.nslice:(c + 1) * sch.nslice]
        flat = np.zeros((sch.nsp, FEAT), np.float32)
        flat[:rows.shape[0]] = rows
        pk = flat.reshape(sch.prows, 2 * FEAT)
        for h in range(2):
            T[h * sch.bankrows + c * sch.halfrows:
              h * sch.bankrows + (c + 1) * sch.halfrows] = \
                pk[h * sch.halfrows:(h + 1) * sch.halfrows]
    out = np.zeros((n_nodes, FEAT), np.float32)
    for c in range(n_cores):
        gidx = sch.core_gidx[c]
        dstrel = sch.core_dstrel[c]
        for s_i in range(len(sch.sgs)):
            rowsb = {}
            for b_i in range(sch.nbank):
                L = sch.gather_L[s_i][b_i]
                go = sch.gidx_off[(s_i, b_i)]
                idx = gidx[:16, go:go + L // 16].T.reshape(-1).astype(np.int64)
                W = T[b_i * sch.bankrows:(b_i + 1) * sch.bankrows]
                rowsb[b_i] = W[idx]
            for (b_i, pr, tc, dcol, blk, st_, sp_) in sch.mmseq[s_i]:
                m = rowsb[b_i][tc * P:(tc + 1) * P, pr * FEAT:(pr + 1) * FEAT]
                S = (dstrel[:, dcol][:, None] ==
                     np.arange(P)[None, :]).astype(np.float32)
                base = c * sch.nslice + blk * P
                hi = min(base + P, n_nodes)
                out[base:hi] += (S.T @ m)[:hi - base]
    return np.abs(out - ref).max() / (np.abs(ref).max() + 1e-9)


# --------------------------------------------------------------------------
# Bass program
# --------------------------------------------------------------------------

def build_program(sch, phases=5, do_compile=True):
    import concourse.mybir as mybir
    import concourse.tile as tile
    from concourse import bacc
    from concourse.masks import make_identity

    dt = mybir.dt
    AF = mybir.ActivationFunctionType
    OP = mybir.AluOpType

    n_cores = sch.n_cores
    nslice, nblk, nbank = sch.nslice, sch.nblk, sch.nbank
    nsp = sch.nsp
    NT = sch.ncols
    subph = os.environ.get("GCN_SUBPH", "full")

    nc = bacc.Bacc("TRN2", target_bir_lowering=False, debug=False,
                   num_devices=n_cores, num_swdge_queues=4)

    xT = nc.dram_tensor("xT", [FEAT, nslice], dt.bfloat16,
                        kind="ExternalInput")
    W1 = nc.dram_tensor("W1", [FEAT, FEAT], dt.bfloat16, kind="ExternalInput")
    W2 = nc.dram_tensor("W2", [FEAT, OUTC], dt.bfloat16, kind="ExternalInput")
    b1r = nc.dram_tensor("b1r", [P, FEAT], dt.float32, kind="ExternalInput")
    b2r = nc.dram_tensor("b2r", [P, OUTC], dt.float32, kind="ExternalInput")
    iota = nc.dram_tensor("iota", [P, P], dt.bfloat16, kind="ExternalInput")
    dinv = nc.dram_tensor("dinv", [P, nblk], dt.float32, kind="ExternalInput")
    gidx = nc.dram_tensor("gidx", [P, max(sch.gidx_cols, 16)], dt.int16,
                          kind="ExternalInput")
    dstrel = nc.dram_tensor("dstrel", [P, NT], dt.bfloat16,
                            kind="ExternalInput")
    zT_out = nc.dram_tensor("zT", [P, nblk * OUTC], dt.float32,
                            kind="ExternalOutput")

    hrows = nsp // 2             # node rows per half-slice
    hblk0 = (nblk + 1) // 2      # first block of half 1
    g0_slice = [nc.dram_tensor(f"g0_slice{h}", [hrows, FEAT], dt.bfloat16)
                for h in range(2)]
    g1_slice = [nc.dram_tensor(f"g1_slice{h}", [hrows, FEAT], dt.bfloat16)
                for h in range(2)]
    g0_full = [nc.dram_tensor(f"g0_full{h}", [n_cores * hrows, FEAT],
                              dt.bfloat16, addr_space="Shared")
               for h in range(2)]
    g1_full = [nc.dram_tensor(f"g1_full{h}", [n_cores * hrows, FEAT],
                              dt.bfloat16, addr_space="Shared")
               for h in range(2)]

    replica_groups = [list(range(n_cores))]
    maxC = sch.maxL // P

    def bank_ap(g_full, b_i):
        """[bankrows, 256B] view of one half's packed node-pair table."""
        return g_full[b_i].ap().rearrange("(r t) f -> r (t f)", t=2)

    def slice_store(g_sl, own, b0, nb):
        """Store own[:, b0:b0+nb, :] into the right half-slice tensor."""
        h = 0 if b0 < hblk0 else 1
        base = (b0 - h * hblk0) * P
        nc.sync.dma_start(
            out=g_sl[h].ap()[base:base + nb * P, :].rearrange(
                "(b p) f -> p b f", p=P),
            in_=own[:, b0:b0 + nb, :])

    def allgather(g_sl, g_fl, h):
        nc.gpsimd.collective_compute(
            "AllGather", OP.bypass, replica_groups=replica_groups,
            ins=[g_sl[h].ap().opt()], outs=[g_fl[h].ap().opt()])

    with tile.TileContext(nc) as tc:
        with (
            tc.tile_pool(name="const", bufs=1) as constp,
            tc.tile_pool(name="gather", bufs=5) as gatherp,
            tc.tile_pool(name="sbuild", bufs=3) as sp,
            tc.tile_pool(name="epi", bufs=3) as epip,
            tc.tile_pool(name="psA", bufs=4, space="PSUM") as psA,
            tc.tile_pool(name="psT", bufs=2, space="PSUM") as psT,
            tc.tile_pool(name="psG", bufs=2, space="PSUM") as psG,
        ):
          try:
            W1_sb = constp.tile([FEAT, FEAT], dt.bfloat16)
            nc.sync.dma_start(out=W1_sb[:], in_=W1.ap())
            W2_sb = constp.tile([FEAT, OUTC], dt.bfloat16)
            nc.sync.dma_start(out=W2_sb[:], in_=W2.ap())
            b1_sb = constp.tile([P, FEAT], dt.float32)
            nc.sync.dma_start(out=b1_sb[:], in_=b1r.ap())
            b2_sb = constp.tile([P, OUTC], dt.float32)
            nc.sync.dma_start(out=b2_sb[:], in_=b2r.ap())
            iota_sb = constp.tile([P, P], dt.bfloat16)
            nc.sync.dma_start(out=iota_sb[:], in_=iota.ap())
            dinv_sb = constp.tile([P, nblk], dt.float32)
            nc.sync.dma_start(out=dinv_sb[:], in_=dinv.ap())
            dstrel_sb = constp.tile([P, NT], dt.bfloat16)
            nc.sync.dma_start(out=dstrel_sb[:], in_=dstrel.ap())
            gidx_sb = constp.tile([P, max(sch.gidx_cols, 16)], dt.int16)
            nc.sync.dma_start(out=gidx_sb[:], in_=gidx.ap())
            ident = constp.tile([P, P], dt.float32)
            make_identity(nc, ident[:])
            own0 = constp.tile([P, nblk, FEAT], dt.bfloat16)
            nc.vector.memset(own0[:], 0.0)
            own1 = constp.tile([P, nblk, FEAT], dt.bfloat16)
            nc.vector.memset(own1[:], 0.0)
            zres = constp.tile([P, nblk, OUTC], dt.float32)

            def blk_rows(blk):
                return min(P, nslice - blk * P)

            # ---------- phase B: own0 = dinv * (x @ W1), bf16 ----------
            for s_i, blks in enumerate(sch.sgs):
                nb = len(blks)
                base = blks[0] * P
                sg_rows = min(nb * P, nslice - base)
                xT_sb = epip.tile([FEAT, SG_BLOCKS * P], dt.bfloat16,
                                  tag="xT")
                nc.sync.dma_start(out=xT_sb[:, :sg_rows],
                                  in_=xT.ap()[:, base:base + sg_rows])
                for j, blk in enumerate(blks):
                    r = blk_rows(blk)
                    ps = psA.tile([P, FEAT], dt.float32, tag="agg")
                    nc.tensor.matmul(ps[:r, :], xT_sb[:, j * P:j * P + r],
                                     W1_sb[:], start=True, stop=True)
                    nc.scalar.mul(own0[:r, blk, :], ps[:r, :],
                                  dinv_sb[:r, blk:blk + 1])
                # store the slice rows for the AllGather
                slice_store(g0_slice, own0, blks[0], nb)
                if blks[-1] == hblk0 - 1:
                    allgather(g0_slice, g0_full, 0)

            if phases < 2:
                raise _PhaseDone()
            allgather(g0_slice, g0_full, 1)
            if phases < 3:
                raise _PhaseDone()

            qn_counter = [0]

            def gather_sg(g_full, s_i, b_i):
                L = sch.gather_L[s_i][b_i]
                if L == 0:
                    return None
                gt = gatherp.tile([P, maxC, 2 * FEAT], dt.bfloat16,
                                  tag="gt")
                src_ap = bank_ap(g_full, b_i)
                go = sch.gidx_off[(s_i, b_i)]
                nch = (L + GMAX - 1) // GMAX
                bsz = (L // (nch * P)) * P
                rem = (L - bsz * nch) // P
                q0 = 0
                for k in range(nch):
                    Lq = bsz + (P if k < rem else 0)
                    q1 = q0 + Lq
                    nc.gpsimd.dma_gather(
                        gt[:, q0 // P:q1 // P, :], src_ap,
                        gidx_sb[:, go + q0 // 16:go + q1 // 16],
                        Lq, Lq, 2 * FEAT,
                        queue_num=qn_counter[0] % 4)
                    qn_counter[0] += 1
                    q0 = q1
                return gt

            def aggregation(layer, g_full, width, epilogue, hooks={}):
                # emit the first two sgs' bank-0 gathers before any bank-1
                # gather: they depend only on the FIRST half-AllGather, so
                # the in-order gpsimd stream keeps working while the second
                # half-AllGather is still in flight.
                pending = {s_i: gather_sg(g_full, s_i, 0)
                           for s_i in range(min(2, len(sch.sgs)))}
                for s_i, blks in enumerate(sch.sgs):
                    gts = {}
                    for b_i in range(nbank):
                        if b_i == 0 and s_i in pending:
                            gts[0] = pending.pop(s_i)
                            continue
                        gt = gather_sg(g_full, s_i, b_i)
                        if gt is not None:
                            gts[b_i] = gt
                    if subph == "gather":
                        continue
                    ps_sg = psA.tile([P, SG_BLOCKS, FEAT], dt.float32,
                                     tag="agg", name=f"agg_l{layer}_{s_i}")
                    sbatch, sb_base = None, -100
                    for (b_i, pr, tc, dcol, blk, st_, sp_) in sch.mmseq[s_i]:
                        if sbatch is None or dcol - sb_base >= SB:
                            w = min(SB, sch.ncols - dcol)
                            sbatch = sp.tile(
                                [P, SB, P],
                                dt.float8e4 if S_FP8 else dt.bfloat16,
                                tag="S")
                            sb_base = dcol
                            nc.vector.tensor_tensor(
                                sbatch[:, :w, :],
                                dstrel_sb[:, dcol:dcol + w, None
                                          ].broadcast_to([P, w, P]),
                                iota_sb[:, None, :].broadcast_to([P, w, P]),
                                OP.is_equal)
                        if subph == "sbuild":
                            continue
                        j = blks.index(blk)
                        S_t = sbatch[:, dcol - sb_base, :]
                        nc.tensor.matmul(
                            ps_sg[:, j, 0:width], S_t,
                            gts[b_i][:, tc, pr * FEAT:pr * FEAT + width],
                            start=st_, stop=sp_)
                    if subph == "full":
                        epilogue(s_i, blks, ps_sg)
                    if s_i in hooks:
                        hooks[s_i]()

            # ---------- layer 1 ----------
            def l1_epilogue(s_i, blks, ps_sg):
                nb = len(blks)
                b0 = blks[0]
                dv = dinv_sb[:, b0:b0 + nb, None].broadcast_to([P, nb, FEAT])
                b1b = b1_sb[:, None, :].broadcast_to([P, nb, FEAT])
                t_sb = epip.tile([P, SG_BLOCKS, FEAT], dt.float32,
                                 tag="tsb", name=f"l1t_{s_i}")
                nc.vector.tensor_tensor(t_sb[:, :nb, :], ps_sg[:, :nb, :],
                                        own0[:, b0:b0 + nb, :], OP.add)
                nc.vector.tensor_tensor(t_sb[:, :nb, :], t_sb[:, :nb, :],
                                        dv, OP.mult)
                nc.vector.tensor_tensor(t_sb[:, :nb, :], t_sb[:, :nb, :],
                                        b1b, OP.add)
                nc.vector.tensor_scalar_max(t_sb[:, :nb, :],
                                            t_sb[:, :nb, :], 0.0)
                nc.vector.tensor_tensor(t_sb[:, :nb, :], t_sb[:, :nb, :],
                                        dv, OP.mult)
                for j, blk in enumerate(blks):
                    r = blk_rows(blk)
                    pst = psT.tile([FEAT, P], dt.float32, tag="pst")
                    nc.tensor.transpose(pst[:, :r], t_sb[:r, j, :],
                                        ident[:r, :r])
                    tT = epip.tile([FEAT, P], dt.bfloat16, tag="tT")
                    nc.scalar.copy(tT[:, :r], pst[:, :r])
                    psg = psG.tile([P, OUTC], dt.float32, tag="psg")
                    nc.tensor.matmul(psg[:r, :], tT[:, :r], W2_sb[:],
                                     start=True, stop=True)
                    nc.scalar.copy(own1[:r, blk, 0:OUTC], psg[:r, :])
                slice_store(g1_slice, own1, b0, nb)

            ag2a_sg = (hblk0 // SG_BLOCKS) - 1
            aggregation(1, g0_full, FEAT, l1_epilogue,
                        hooks={ag2a_sg: lambda: allgather(
                            g1_slice, g1_full, 0)})

            if phases < 4:
                raise _PhaseDone()
            allgather(g1_slice, g1_full, 1)
            if phases < 5:
                raise _PhaseDone()

            # ---------- layer 2 + head ----------
            def l2_epilogue(s_i, blks, ps_sg):
                nb = len(blks)
                b0 = blks[0]
                dv = dinv_sb[:, b0:b0 + nb, None].broadcast_to([P, nb, OUTC])
                b2b = b2_sb[:, None, :].broadcast_to([P, nb, OUTC])
                z_sb = epip.tile([P, SG_BLOCKS, OUTC], dt.float32,
                                 tag="zsb", name=f"z_{s_i}")
                nc.vector.tensor_tensor(z_sb[:, :nb, :],
                                        ps_sg[:, :nb, 0:OUTC],
                                        own1[:, b0:b0 + nb, 0:OUTC], OP.add)
                nc.vector.tensor_tensor(z_sb[:, :nb, :], z_sb[:, :nb, :],
                                        dv, OP.mult)
                nc.vector.tensor_tensor(z_sb[:, :nb, :], z_sb[:, :nb, :],
                                        b2b, OP.add)
                mx = epip.tile([P, SG_BLOCKS], dt.float32, tag="mx")
                nc.vector.tensor_reduce(
                    mx[:, :nb], z_sb[:, :nb, :],
                    axis=mybir.AxisListType.X, op=OP.max)
                mxb = mx[:, :nb, None].broadcast_to([P, nb, OUTC])
                nc.vector.tensor_tensor(z_sb[:, :nb, :], z_sb[:, :nb, :],
                                        mxb, OP.subtract)
                ex = epip.tile([P, SG_BLOCKS, OUTC], dt.float32, tag="ex")
                nc.scalar.activation(ex[:, :nb, :], z_sb[:, :nb, :], AF.Exp)
                sm = epip.tile([P, SG_BLOCKS], dt.float32, tag="sm")
                nc.vector.tensor_reduce(
                    sm[:, :nb], ex[:, :nb, :],
                    axis=mybir.AxisListType.X, op=OP.add)
                lse = epip.tile([P, SG_BLOCKS], dt.float32, tag="lse")
                nc.scalar.activation(lse[:, :nb], sm[:, :nb], AF.Ln)
                lseb = lse[:, :nb, None].broadcast_to([P, nb, OUTC])
                nc.vector.tensor_tensor(zres[:, b0:b0 + nb, :],
                                        z_sb[:, :nb, :], lseb, OP.subtract)

            aggregation(2, g1_full, OUTC, l2_epilogue)

            nc.sync.dma_start(
                out=zT_out.ap().rearrange("p (b f) -> p b f", b=nblk),
                in_=zres[:])
          except _PhaseDone:
            pass

    if do_compile:
        nc.compile()
    return nc


# --------------------------------------------------------------------------
# Entry point
# --------------------------------------------------------------------------

_cache = {}


def _to_bf16(a):
    import ml_dtypes
    return np.asarray(a).astype(ml_dtypes.bfloat16)


def make_in_maps(sch, x, dst, W1, b1, W2, b2):
    n_nodes = sch.n_nodes
    deg = np.bincount(dst, minlength=n_nodes).astype(np.float32) + 1.0
    dinv = 1.0 / np.sqrt(deg)
    nslice, nblk = sch.nslice, sch.nblk
    in_maps = []
    iota_np = np.tile(np.arange(P, dtype=np.float32)[None, :], (P, 1))
    for c in range(sch.n_cores):
        xs = np.asarray(x[c * nslice:(c + 1) * nslice], np.float32)
        dv = dinv[c * nslice:(c + 1) * nslice]
        dv_pad = np.ones(nblk * P, np.float32)
        dv_pad[:nslice] = dv
        in_maps.append({
            "xT": _to_bf16(np.ascontiguousarray(xs.T)),
            "W1": _to_bf16(W1),
            "W2": _to_bf16(W2),
            "b1r": np.tile(np.asarray(b1, np.float32)[None, :], (P, 1)),
            "b2r": np.tile(np.asarray(b2, np.float32)[None, :], (P, 1)),
            "iota": _to_bf16(iota_np),
            "dinv": np.ascontiguousarray(dv_pad.reshape(nblk, P).T),
            "gidx": sch.core_gidx[c],
            "dstrel": _to_bf16(sch.core_dstrel[c]),
        })
    return in_maps


def _ensure_ntff_hook():
    import types
    try:
        from antenv import axon_hooks  # noqa: F401
        return
    except ImportError:
        pass
    try:
        from trn_agent_boot.trn_boot import _ntff_profile_via_ctypes
        hook = _ntff_profile_via_ctypes("/opt/axon/libaxon_pjrt.so")
        m = types.ModuleType("antenv.axon_hooks")
        m.get_axon_ntff_profile_hook = lambda: hook
        m.set_axon_ntff_profile_hook = lambda h: None
        sys.modules["antenv.axon_hooks"] = m
    except Exception:
        pass


def kernel(x, edge_index, W1, b1, W2, b2):
    _phases = int(os.environ.get("GCN_PHASES", "5"))
    x = np.asarray(x)
    edge_index = np.asarray(edge_index)
    n_nodes = x.shape[0]
    n_cores = 8
    src = edge_index[0].astype(np.int64)
    dst = edge_index[1].astype(np.int64)

    ck = (n_nodes, edge_index.shape[1],
          int(edge_index[:, :100].sum()), int(edge_index[:, -100:].sum()))
    if ck in _cache:
        sch, nc = _cache[ck]
    else:
        sch = build_schedule(src, dst, n_nodes, n_cores)
        nc = build_program(sch, phases=_phases)
        _cache[ck] = (sch, nc)

    in_maps = make_in_maps(sch, x, dst, W1, b1, W2, b2)

    from concourse.bass_utils import run_bass_kernel_spmd
    trace = bool(int(os.environ.get("GCN_TRACE", "0")))
    if trace:
        _ensure_ntff_hook()
    try:
        res = run_bass_kernel_spmd(nc, in_maps, core_ids=list(range(n_cores)),
                                   trace=trace)
    except Exception:
        if not trace:
            raise
        res = run_bass_kernel_spmd(nc, in_maps, core_ids=list(range(n_cores)),
                                   trace=False)
    kernel._last_results = res
    nblk = sch.nblk
    outs = []
    for c in range(n_cores):
        zT = np.asarray(res.results[c]["zT"], np.float32)
        z = zT.reshape(P, nblk, OUTC).transpose(1, 0, 2).reshape(
            nblk * P, OUTC)[:sch.nslice]
        outs.append(z)
    return np.ascontiguousarray(np.concatenate(outs, axis=0))


if __name__ == "__main__":
    rng = np.random.default_rng(0)
    N, E = 4096, 60000
    src = rng.integers(0, N, E)
    dst = rng.integers(0, N, E)
    sch = build_schedule(src, dst, N, 8)
    print("ncols", sch.ncols, "gidx_cols", sch.gidx_cols, "maxL", sch.maxL)
    print("schedule numpy check rel err:",
          numpy_check_schedule(sch, src, dst, N))
    # full-size schedule stats
    src = rng.integers(0, 100000, 1600000)
    dst = rng.integers(0, 100000, 1600000)
    sch = build_schedule(src, dst, 100000, 8)
    tot_slots = sum(sum(Ls) for Ls in sch.gather_L)
    gathers = sum((L + GMAX - 1) // GMAX
                  for Ls in sch.gather_L for L in Ls)
    print(f"full: ncols={sch.ncols} slots/layer={tot_slots} "
          f"gathers/layer={gathers} maxL={sch.maxL} gidx_cols={sch.gidx_cols}")


# revision 18
# speedup vs baseline: 1.0799x; 1.0799x over previous
"""Trainium2 Bass kernel for a 2-layer GCN (GCNConv -> ReLU -> GCNConv -> log_softmax).

Strategy (8 NeuronCores, SPMD, one NEFF):
  * Nodes range-sharded by destination; each core owns N/8 dst nodes and all
    edges pointing at them.  out = D^-1/2 (A+I) D^-1/2 h factorizes: tables
    hold g = dinv * h; epilogue does (agg + g_own) * dinv + b.
  * Layer-2 applies W2 EARLY:  A(H1 W2) = (A H1) W2, so the layer-2 table is
    g1 = dinv*(relu(l1) @ W2) [N, 32] and layer-2 agg matmuls are 32 wide.
  * Tables are bf16, 128 B per node (64 feats; layer2: 32 feats + 32 zero
    pad).  dma_gather rows must be 256 B, so gathers fetch NODE PAIRS:
    idx = src//2 and the one-hot S matmul reads the parity half
    (rhs columns parity*64 + [0..W)).  Runs are parity-pure (host-sorted),
    and slots within each run are sorted by source row (HBM locality).
  * 2 bank windows of 4 cores (25088 packed rows < int16 range); runs per
    dst block = 2 banks x 2 parities = 4.  The SAME gidx + dstrel data
    serve BOTH layers (identical edge structure, only the table differs).
  * gidx lives resident in SBUF (one load); no per-gather index DMAs.
  * Own-slice tables (self-loop terms) live resident in SBUF (own0/own1).
  * One-hot S matrices are built on DVE in fp8e4 and feed the PE as the
    fp8 stationary against bf16 moving messages (mixed-dtype matmul).
  * Per-layer AllGather of the 1.6 MB bf16 slice (~60 us each).

Measured on HW: 1.349 ms (baseline this replaced: 1.92 ms).  Bottleneck is
the gpsimd SWDGE descriptor generation for the per-edge gathers (~2.5 us
per 1024-idx dma_gather, ~82% engine busy); the 1024-desc carveout is a
hard runtime limit (bigger instructions hang await_space), so per-edge
descgen cost is the floor of this design.
"""

import os
import sys
import numpy as np

P = 128
FEAT = 64
OUTC = 32
SG_BLOCKS = 7
GMAX = 1024      # max idxs per dma_gather instruction. HARD LIMIT: the
                 # SWDGE descriptor carveout is fixed at 1024 descs by the
                 # runtime; 1984/2048-idx instructions hang await_space
                 # (tested on reset devices, dynamic_dma_scratch_size has
                 # no effect).
SB = 64          # S-build batch (dcols per is_equal op)
S_FP8 = True     # one-hot S matrices in fp8e4 (stationary side of the
                 # segment-sum matmuls; moving side stays bf16)


class _PhaseDone(Exception):
    pass


class Schedule:
    pass


# --------------------------------------------------------------------------
# Host-side schedule construction
# --------------------------------------------------------------------------

def build_schedule(src, dst, n_nodes, n_cores):
    """Edges grouped per (dst core, dst 128-block, src bank-window, src
    parity); run lengths 16-quantized at the max over cores (SPMD shared
    schedule).  Gather idx = packed node-pair row within the bank window."""
    Q = 1    # run offsets need no alignment (gidx wrap only needs L%128==0)
    nslice = n_nodes // n_cores
    nblk = (nslice + P - 1) // P
    nsp = nblk * P                    # padded slice rows (node granularity)
    prows = nsp // 2                  # packed pair-rows per core
    nbank = 2
    halfrows = prows // 2             # packed rows per half-slice
    bankrows = n_cores * halfrows     # bank window = all cores' half-slices
    assert bankrows <= 32767

    src_a = src.astype(np.int64)
    dst_a = dst.astype(np.int64)
    core = dst_a // nslice
    block = (dst_a % nslice) // P
    sc = src_a // nslice
    sl = src_a % nslice
    bank = sl // (2 * halfrows)       # which half of the owner's slice
    par = sl % 2
    pidx = sc * halfrows + (sl // 2) % halfrows

    key = ((core * nblk + block) * nbank + bank) * 2 + par
    counts = np.bincount(key, minlength=n_cores * nblk * nbank * 2).reshape(
        n_cores, nblk, nbank, 2)
    R = (np.ceil(counts.max(axis=0) / Q) * Q).astype(np.int64)
    for b in range(nblk):
        if R[b].sum() == 0:
            R[b, 0, 0] = Q

    sgs = [list(range(i, min(i + SG_BLOCKS, nblk)))
           for i in range(0, nblk, SG_BLOCKS)]

    sch = Schedule()
    sch.n_nodes, sch.n_cores, sch.nslice = n_nodes, n_cores, nslice
    sch.nblk, sch.nsp, sch.prows = nblk, nsp, prows
    sch.nbank, sch.bankrows, sch.halfrows = nbank, bankrows, halfrows
    sch.sgs = sgs
    sch.R = R

    # per (sg, bank): run offsets and padded gather length
    sch.run_off = []     # [s_i][(blk, b, par)] -> slot offset
    sch.gather_L = []    # [s_i][b] -> padded length (128-multiple)
    for s_i, blks in enumerate(sgs):
        offs = {}
        Ls = []
        for b_i in range(nbank):
            o = 0
            for blk in blks:
                for pr in range(2):
                    offs[(blk, b_i, pr)] = o
                    o += int(R[blk, b_i, pr])
            o = ((o + P - 1) // P) * P
            Ls.append(o)
        sch.run_off.append(offs)
        sch.gather_L.append(Ls)
    sch.maxL = max(max(Ls) for Ls in sch.gather_L)

    # matmul sequence: per sg, block-major; entries are run-tile overlaps.
    # Entry: (b_i, par, tc, dcol, blk, start, stop)
    sch.mmseq = []
    ncols = 0
    dcol_map = []
    for s_i, blks in enumerate(sgs):
        seq = []
        for blk in blks:
            lst = []
            for b_i in range(nbank):
                for pr in range(2):
                    r0 = sch.run_off[s_i][(blk, b_i, pr)]
                    r1 = r0 + int(R[blk, b_i, pr])
                    if r1 == r0:
                        continue
                    t0, t1 = r0 // P, (r1 - 1) // P
                    for tc in range(t0, t1 + 1):
                        lst.append((b_i, pr, tc))
            for i, (b_i, pr, tc) in enumerate(lst):
                seq.append([b_i, pr, tc, ncols, blk, i == 0,
                            i == len(lst) - 1])
                dcol_map.append((s_i, b_i, pr, tc, blk))
                ncols += 1
        sch.mmseq.append(seq)
    sch.ncols = ncols
    sch.dcol_map = dcol_map

    # gidx column layout
    off = 0
    gidx_off = {}
    for s_i in range(len(sgs)):
        for b_i in range(nbank):
            gidx_off[(s_i, b_i)] = off
            off += sch.gather_L[s_i][b_i] // 16
    sch.gidx_cols = off
    sch.gidx_off = gidx_off

    # ---------------- per-core arrays ----------------
    # pidx as the fastest key: slots within each run come sorted by source
    # row, so gather descriptor streams walk increasing HBM addresses.
    order = np.lexsort((pidx, par, bank, block, core))
    p_o = pidx[order]
    d_o = dst_a[order]
    grp_key = key[order]
    uniq, starts = np.unique(grp_key, return_index=True)
    grp_start = {int(k): int(v) for k, v in zip(uniq, starts)}
    grp_count = {int(k): int(v) for k, v in
                 zip(uniq, np.diff(np.append(starts, len(grp_key))))}

    sch.core_gidx = []
    sch.core_dstrel = []
    for c in range(n_cores):
        gidx = np.zeros((16, max(sch.gidx_cols, 16)), dtype=np.int16)
        dstrel = np.full((P, sch.ncols), -1.0, dtype=np.float32)
        slot_dst = {}
        for s_i, blks in enumerate(sgs):
            for b_i in range(nbank):
                L = sch.gather_L[s_i][b_i]
                idx_lin = np.zeros(L, dtype=np.int16)
                dst_lin = np.full(L, -1, dtype=np.int64)
                for blk in blks:
                    for pr in range(2):
                        k = int((((c * nblk + blk) * nbank + b_i) * 2 + pr))
                        cnt = grp_count.get(k, 0)
                        if not cnt:
                            continue
                        st = grp_start[k]
                        o = sch.run_off[s_i][(blk, b_i, pr)]
                        idx_lin[o:o + cnt] = p_o[st:st + cnt].astype(np.int16)
                        dst_lin[o:o + cnt] = d_o[st:st + cnt]
                go = gidx_off[(s_i, b_i)]
                gidx[:, go:go + L // 16] = idx_lin.reshape(L // 16, 16).T
                slot_dst[(s_i, b_i)] = dst_lin
        for dcol, (s_i, b_i, pr, tc, blk) in enumerate(dcol_map):
            dl = slot_dst.get((s_i, b_i))
            if dl is None:
                continue
            r0 = sch.run_off[s_i][(blk, b_i, pr)]
            r1 = r0 + int(sch.R[blk, b_i, pr])
            lo = max(r0, tc * P)
            hi = min(r1, (tc + 1) * P)
            if hi <= lo:
                continue
            seg = dl[lo:hi]
            base = c * nslice + blk * P
            vals = seg - base
            vals = np.where((seg >= 0) & (vals >= 0) & (vals < P),
                            vals, -1).astype(np.float32)
            dstrel[lo - tc * P:hi - tc * P, dcol] = vals
        sch.core_gidx.append(np.tile(gidx, (8, 1)))
        sch.core_dstrel.append(dstrel)
    return sch


def numpy_check_schedule(sch, src, dst, n_nodes):
    """Emulate the device aggregation (no self loops) in numpy, both widths."""
    rng = np.random.default_rng(0)
    n_cores = sch.n_cores
    g = rng.standard_normal((n_nodes, FEAT)).astype(np.float32)
    ref = np.zeros((n_nodes, FEAT), np.float32)
    np.add.at(ref, dst, g[src])
    # build the packed half-major table [2 * bankrows, 128]
    T = np.zeros((2 * sch.bankrows, 2 * FEAT), np.float32)
    for c in range(n_cores):
        rows = g[c * sch.nslice:(c + 1) * sch.nslice]
        flat = np.zeros((sch.nsp, FEAT), np.float32)
        flat[:rows.shape[0]] = rows
        pk = flat.reshape(sch.prows, 2 * FEAT)
        for h in range(2):
            T[h * sch.bankrows + c * sch.halfrows:
              h * sch.bankrows + (c + 1) * sch.halfrows] = \
                pk[h * sch.halfrows:(h + 1) * sch.halfrows]
    out = np.zeros((n_nodes, FEAT), np.float32)
    for c in range(n_cores):
        gidx = sch.core_gidx[c]
        dstrel = sch.core_dstrel[c]
        for s_i in range(len(sch.sgs)):
            rowsb = {}
            for b_i in range(sch.nbank):
                L = sch.gather_L[s_i][b_i]
                go = sch.gidx_off[(s_i, b_i)]
                idx = gidx[:16, go:go + L // 16].T.reshape(-1).astype(np.int64)
                W = T[b_i * sch.bankrows:(b_i + 1) * sch.bankrows]
                rowsb[b_i] = W[idx]
            for (b_i, pr, tc, dcol, blk, st_, sp_) in sch.mmseq[s_i]:
                m = rowsb[b_i][tc * P:(tc + 1) * P, pr * FEAT:(pr + 1) * FEAT]
                S = (dstrel[:, dcol][:, None] ==
                     np.arange(P)[None, :]).astype(np.float32)
                base = c * sch.nslice + blk * P
                hi = min(base + P, n_nodes)
                out[base:hi] += (S.T @ m)[:hi - base]
    return np.abs(out - ref).max() / (np.abs(ref).max() + 1e-9)


# --------------------------------------------------------------------------
# Bass program
# --------------------------------------------------------------------------

def build_program(sch, phases=5, do_compile=True):
    import concourse.mybir as mybir
    import concourse.tile as tile
    from concourse import bacc
    from concourse.masks import make_identity

    dt = mybir.dt
    AF = mybir.ActivationFunctionType
    OP = mybir.AluOpType

    n_cores = sch.n_cores
    nslice, nblk, nbank = sch.nslice, sch.nblk, sch.nbank
    nsp = sch.nsp
    NT = sch.ncols
    subph = os.environ.get("GCN_SUBPH", "full")

    nc = bacc.Bacc("TRN2", target_bir_lowering=False, debug=False,
                   num_devices=n_cores, num_swdge_queues=4)

    xT = nc.dram_tensor("xT", [FEAT, nslice], dt.bfloat16,
                        kind="ExternalInput")
    W1 = nc.dram_tensor("W1", [FEAT, FEAT], dt.bfloat16, kind="ExternalInput")
    W2 = nc.dram_tensor("W2", [FEAT, OUTC], dt.bfloat16, kind="ExternalInput")
    b1r = nc.dram_tensor("b1r", [P, FEAT], dt.float32, kind="ExternalInput")
    b2r = nc.dram_tensor("b2r", [P, OUTC], dt.float32, kind="ExternalInput")
    iota = nc.dram_tensor("iota", [P, P], dt.bfloat16, kind="ExternalInput")
    dinv = nc.dram_tensor("dinv", [P, nblk], dt.float32, kind="ExternalInput")
    gidx = nc.dram_tensor("gidx", [P, max(sch.gidx_cols, 16)], dt.int16,
                          kind="ExternalInput")
    dstrel = nc.dram_tensor("dstrel", [P, NT], dt.bfloat16,
                            kind="ExternalInput")
    zT_out = nc.dram_tensor("zT", [P, nblk * OUTC], dt.float32,
                            kind="ExternalOutput")

    hrows = nsp // 2             # node rows per half-slice
    hblk0 = (nblk + 1) // 2      # first block of half 1
    g0_slice = [nc.dram_tensor(f"g0_slice{h}", [hrows, FEAT], dt.bfloat16)
                for h in range(2)]
    g1_slice = [nc.dram_tensor(f"g1_slice{h}", [hrows, FEAT], dt.bfloat16)
                for h in range(2)]
    g0_full = [nc.dram_tensor(f"g0_full{h}", [n_cores * hrows, FEAT],
                              dt.bfloat16, addr_space="Shared")
               for h in range(2)]
    g1_full = [nc.dram_tensor(f"g1_full{h}", [n_cores * hrows, FEAT],
                              dt.bfloat16, addr_space="Shared")
               for h in range(2)]

    replica_groups = [list(range(n_cores))]
    maxC = sch.maxL // P

    def bank_ap(g_full, b_i):
        """[bankrows, 256B] view of one half's packed node-pair table."""
        return g_full[b_i].ap().rearrange("(r t) f -> r (t f)", t=2)

    def slice_store(g_sl, own, b0, nb):
        """Store own[:, b0:b0+nb, :] into the right half-slice tensor."""
        h = 0 if b0 < hblk0 else 1
        base = (b0 - h * hblk0) * P
        nc.sync.dma_start(
            out=g_sl[h].ap()[base:base + nb * P, :].rearrange(
                "(b p) f -> p b f", p=P),
            in_=own[:, b0:b0 + nb, :])

    def allgather(g_sl, g_fl, h):
        nc.gpsimd.collective_compute(
            "AllGather", OP.bypass, replica_groups=replica_groups,
            ins=[g_sl[h].ap().opt()], outs=[g_fl[h].ap().opt()])

    with tile.TileContext(nc) as tc:
        with (
            tc.tile_pool(name="const", bufs=1) as constp,
            tc.tile_pool(name="gather", bufs=3) as gatherp,
            tc.tile_pool(name="sbuild", bufs=3) as sp,
            tc.tile_pool(name="epi", bufs=3) as epip,
            tc.tile_pool(name="psA", bufs=4, space="PSUM") as psA,
            tc.tile_pool(name="psT", bufs=2, space="PSUM") as psT,
            tc.tile_pool(name="psG", bufs=2, space="PSUM") as psG,
        ):
          try:
            W1_sb = constp.tile([FEAT, FEAT], dt.bfloat16)
            nc.sync.dma_start(out=W1_sb[:], in_=W1.ap())
            W2_sb = constp.tile([FEAT, OUTC], dt.bfloat16)
            nc.sync.dma_start(out=W2_sb[:], in_=W2.ap())
            b1_sb = constp.tile([P, FEAT], dt.float32)
            nc.sync.dma_start(out=b1_sb[:], in_=b1r.ap())
            b2_sb = constp.tile([P, OUTC], dt.float32)
            nc.sync.dma_start(out=b2_sb[:], in_=b2r.ap())
            iota_sb = constp.tile([P, P], dt.bfloat16)
            nc.sync.dma_start(out=iota_sb[:], in_=iota.ap())
            dinv_sb = constp.tile([P, nblk], dt.float32)
            nc.sync.dma_start(out=dinv_sb[:], in_=dinv.ap())
            dstrel_sb = constp.tile([P, NT], dt.bfloat16)
            nc.sync.dma_start(out=dstrel_sb[:], in_=dstrel.ap())
            gidx_sb = constp.tile([P, max(sch.gidx_cols, 16)], dt.int16)
            nc.sync.dma_start(out=gidx_sb[:], in_=gidx.ap())
            ident = constp.tile([P, P], dt.float32)
            make_identity(nc, ident[:])
            own0 = constp.tile([P, nblk, FEAT], dt.bfloat16)
            nc.vector.memset(own0[:], 0.0)
            own1 = constp.tile([P, nblk, FEAT], dt.bfloat16)
            nc.vector.memset(own1[:], 0.0)
            zres = constp.tile([P, nblk, OUTC], dt.float32)

            def blk_rows(blk):
                return min(P, nslice - blk * P)

            # ---------- phase B: own0 = dinv * (x @ W1), bf16 ----------
            for s_i, blks in enumerate(sch.sgs):
                nb = len(blks)
                base = blks[0] * P
                sg_rows = min(nb * P, nslice - base)
                xT_sb = epip.tile([FEAT, SG_BLOCKS * P], dt.bfloat16,
                                  tag="xT")
                nc.sync.dma_start(out=xT_sb[:, :sg_rows],
                                  in_=xT.ap()[:, base:base + sg_rows])
                for j, blk in enumerate(blks):
                    r = blk_rows(blk)
                    ps = psA.tile([P, FEAT], dt.float32, tag="agg")
                    nc.tensor.matmul(ps[:r, :], xT_sb[:, j * P:j * P + r],
                                     W1_sb[:], start=True, stop=True)
                    nc.scalar.mul(own0[:r, blk, :], ps[:r, :],
                                  dinv_sb[:r, blk:blk + 1])
                # store the slice rows for the AllGather
                slice_store(g0_slice, own0, blks[0], nb)
                if blks[-1] == hblk0 - 1:
                    allgather(g0_slice, g0_full, 0)

            if phases < 2:
                raise _PhaseDone()
            allgather(g0_slice, g0_full, 1)
            if phases < 3:
                raise _PhaseDone()

            qn_counter = [0]

            def gather_sg(g_full, s_i, b_i):
                L = sch.gather_L[s_i][b_i]
                if L == 0:
                    return None
                gt = gatherp.tile([P, maxC, 2 * FEAT], dt.bfloat16,
                                  tag="gt")
                src_ap = bank_ap(g_full, b_i)
                go = sch.gidx_off[(s_i, b_i)]
                nch = (L + GMAX - 1) // GMAX
                bsz = (L // (nch * P)) * P
                rem = (L - bsz * nch) // P
                q0 = 0
                for k in range(nch):
                    Lq = bsz + (P if k < rem else 0)
                    q1 = q0 + Lq
                    nc.gpsimd.dma_gather(
                        gt[:, q0 // P:q1 // P, :], src_ap,
                        gidx_sb[:, go + q0 // 16:go + q1 // 16],
                        Lq, Lq, 2 * FEAT,
                        queue_num=qn_counter[0] % 4)
                    qn_counter[0] += 1
                    q0 = q1
                return gt

            def aggregation(layer, g_full, width, epilogue, hooks={}):
                for s_i, blks in enumerate(sch.sgs):
                    gts = {}
                    for b_i in range(nbank):
                        gt = gather_sg(g_full, s_i, b_i)
                        if gt is not None:
                            gts[b_i] = gt
                    if subph == "gather":
                        continue
                    ps_sg = psA.tile([P, SG_BLOCKS, FEAT], dt.float32,
                                     tag="agg", name=f"agg_l{layer}_{s_i}")
                    sbatch, sb_base = None, -100
                    for (b_i, pr, tc, dcol, blk, st_, sp_) in sch.mmseq[s_i]:
                        if sbatch is None or dcol - sb_base >= SB:
                            w = min(SB, sch.ncols - dcol)
                            sbatch = sp.tile(
                                [P, SB, P],
                                dt.float8e4 if S_FP8 else dt.bfloat16,
                                tag="S")
                            sb_base = dcol
                            nc.vector.tensor_tensor(
                                sbatch[:, :w, :],
                                dstrel_sb[:, dcol:dcol + w, None
                                          ].broadcast_to([P, w, P]),
                                iota_sb[:, None, :].broadcast_to([P, w, P]),
                                OP.is_equal)
                        if subph == "sbuild":
                            continue
                        j = blks.index(blk)
                        S_t = sbatch[:, dcol - sb_base, :]
                        nc.tensor.matmul(
                            ps_sg[:, j, 0:width], S_t,
                            gts[b_i][:, tc, pr * FEAT:pr * FEAT + width],
                            start=st_, stop=sp_)
                    if subph == "full":
                        epilogue(s_i, blks, ps_sg)
                    if s_i in hooks:
                        hooks[s_i]()

            # ---------- layer 1 ----------
            def l1_epilogue(s_i, blks, ps_sg):
                nb = len(blks)
                b0 = blks[0]
                dv = dinv_sb[:, b0:b0 + nb, None].broadcast_to([P, nb, FEAT])
                b1b = b1_sb[:, None, :].broadcast_to([P, nb, FEAT])
                t_sb = epip.tile([P, SG_BLOCKS, FEAT], dt.float32,
                                 tag="tsb", name=f"l1t_{s_i}")
                nc.vector.tensor_tensor(t_sb[:, :nb, :], ps_sg[:, :nb, :],
                                        own0[:, b0:b0 + nb, :], OP.add)
                nc.vector.tensor_tensor(t_sb[:, :nb, :], t_sb[:, :nb, :],
                                        dv, OP.mult)
                nc.vector.tensor_tensor(t_sb[:, :nb, :], t_sb[:, :nb, :],
                                        b1b, OP.add)
                nc.scalar.activation(t_sb[:, :nb, :], t_sb[:, :nb, :],
                                     AF.Relu)
                nc.vector.tensor_tensor(t_sb[:, :nb, :], t_sb[:, :nb, :],
                                        dv, OP.mult)
                for j, blk in enumerate(blks):
                    r = blk_rows(blk)
                    pst = psT.tile([FEAT, P], dt.float32, tag="pst")
                    nc.tensor.transpose(pst[:, :r], t_sb[:r, j, :],
                                        ident[:r, :r])
                    tT = epip.tile([FEAT, P], dt.bfloat16, tag="tT")
                    nc.scalar.copy(tT[:, :r], pst[:, :r])
                    psg = psG.tile([P, OUTC], dt.float32, tag="psg")
                    nc.tensor.matmul(psg[:r, :], tT[:, :r], W2_sb[:],
                                     start=True, stop=True)
                    nc.scalar.copy(own1[:r, blk, 0:OUTC], psg[:r, :])
                slice_store(g1_slice, own1, b0, nb)

            ag2a_sg = (hblk0 // SG_BLOCKS) - 1
            aggregation(1, g0_full, FEAT, l1_epilogue,
                        hooks={ag2a_sg: lambda: allgather(
                            g1_slice, g1_full, 0)})

            if phases < 4:
                raise _PhaseDone()
            allgather(g1_slice, g1_full, 1)
            if phases < 5:
                raise _PhaseDone()

            # ---------- layer 2 + head ----------
            def l2_epilogue(s_i, blks, ps_sg):
                nb = len(blks)
                b0 = blks[0]
                dv = dinv_sb[:, b0:b0 + nb, None].broadcast_to([P, nb, OUTC])
                b2b = b2_sb[:, None, :].broadcast_to([P, nb, OUTC])
                z_sb = epip.tile([P, SG_BLOCKS, OUTC], dt.float32,
                                 tag="zsb", name=f"z_{s_i}")
                nc.vector.tensor_tensor(z_sb[:, :nb, :],
                                        ps_sg[:, :nb, 0:OUTC],
                                        own1[:, b0:b0 + nb, 0:OUTC], OP.add)
                nc.vector.tensor_tensor(z_sb[:, :nb, :], z_sb[:, :nb, :],
                                        dv, OP.mult)
                nc.vector.tensor_tensor(z_sb[:, :nb, :], z_sb[:, :nb, :],
                                        b2b, OP.add)
                mx = epip.tile([P, SG_BLOCKS], dt.float32, tag="mx")
                nc.vector.tensor_reduce(
                    mx[:, :nb], z_sb[:, :nb, :],
                    axis=mybir.AxisListType.X, op=OP.max)
                mxb = mx[:, :nb, None].broadcast_to([P, nb, OUTC])
                nc.vector.tensor_tensor(z_sb[:, :nb, :], z_sb[:, :nb, :],
                                        mxb, OP.subtract)
                ex = epip.tile([P, SG_BLOCKS, OUTC], dt.float32, tag="ex")
                nc.scalar.activation(ex[:, :nb, :], z_sb[:, :nb, :], AF.Exp)
                sm = epip.tile([P, SG_BLOCKS], dt.float32, tag="sm")
                nc.vector.tensor_reduce(
                    sm[:, :nb], ex[:, :nb, :],
                    axis=mybir.AxisListType.X, op=OP.add)
                lse = epip.tile([P, SG_BLOCKS], dt.float32, tag="lse")
                nc.scalar.activation(lse[:, :nb], sm[:, :nb], AF.Ln)
                lseb = lse[:, :nb, None].broadcast_to([P, nb, OUTC])
                nc.vector.tensor_tensor(zres[:, b0:b0 + nb, :],
                                        z_sb[:, :nb, :], lseb, OP.subtract)

            aggregation(2, g1_full, OUTC, l2_epilogue)

            nc.sync.dma_start(
                out=zT_out.ap().rearrange("p (b f) -> p b f", b=nblk),
                in_=zres[:])
          except _PhaseDone:
            pass

    if do_compile:
        nc.compile()
    return nc


# --------------------------------------------------------------------------
# Entry point
# --------------------------------------------------------------------------

_cache = {}


def _to_bf16(a):
    import ml_dtypes
    return np.asarray(a).astype(ml_dtypes.bfloat16)


def make_in_maps(sch, x, dst, W1, b1, W2, b2):
    n_nodes = sch.n_nodes
    deg = np.bincount(dst, minlength=n_nodes).astype(np.float32) + 1.0
    dinv = 1.0 / np.sqrt(deg)
    nslice, nblk = sch.nslice, sch.nblk
    in_maps = []
    iota_np = np.tile(np.arange(P, dtype=np.float32)[None, :], (P, 1))
    for c in range(sch.n_cores):
        xs = np.asarray(x[c * nslice:(c + 1) * nslice], np.float32)
        dv = dinv[c * nslice:(c + 1) * nslice]
        dv_pad = np.ones(nblk * P, np.float32)
        dv_pad[:nslice] = dv
        in_maps.append({
            "xT": _to_bf16(np.ascontiguousarray(xs.T)),
            "W1": _to_bf16(W1),
            "W2": _to_bf16(W2),
            "b1r": np.tile(np.asarray(b1, np.float32)[None, :], (P, 1)),
            "b2r": np.tile(np.asarray(b2, np.float32)[None, :], (P, 1)),
            "iota": _to_bf16(iota_np),
            "dinv": np.ascontiguousarray(dv_pad.reshape(nblk, P).T),
            "gidx": sch.core_gidx[c],
            "dstrel": _to_bf16(sch.core_dstrel[c]),
        })
    return in_maps


def _ensure_ntff_hook():
    import types
    try:
        from antenv import axon_hooks  # noqa: F401
        return
    except ImportError:
        pass
    try:
        from trn_agent_boot.trn_boot import _ntff_profile_via_ctypes
        hook = _ntff_profile_via_ctypes("/opt/axon/libaxon_pjrt.so")
        m = types.ModuleType("antenv.axon_hooks")
        m.get_axon_ntff_profile_hook = lambda: hook
        m.set_axon_ntff_profile_hook = lambda h: None
        sys.modules["antenv.axon_hooks"] = m
    except Exception:
        pass


def kernel(x, edge_index, W1, b1, W2, b2):
    _phases = int(os.environ.get("GCN_PHASES", "5"))
    x = np.asarray(x)
    edge_index = np.asarray(edge_index)
    n_nodes = x.shape[0]
    n_cores = 8
    src = edge_index[0].astype(np.int64)
    dst = edge_index[1].astype(np.int64)

    ck = (n_nodes, edge_index.shape[1],
          int(edge_index[:, :100].sum()), int(edge_index[:, -100:].sum()))
    if ck in _cache:
        sch, nc = _cache[ck]
    else:
        sch = build_schedule(src, dst, n_nodes, n_cores)
        nc = build_program(sch, phases=_phases)
        _cache[ck] = (sch, nc)

    in_maps = make_in_maps(sch, x, dst, W1, b1, W2, b2)

    from concourse.bass_utils import run_bass_kernel_spmd
    trace = bool(int(os.environ.get("GCN_TRACE", "0")))
    if trace:
        _ensure_ntff_hook()
    try:
        res = run_bass_kernel_spmd(nc, in_maps, core_ids=list(range(n_cores)),
                                   trace=trace)
    except Exception:
        if not trace:
            raise
        res = run_bass_kernel_spmd(nc, in_maps, core_ids=list(range(n_cores)),
                                   trace=False)
    kernel._last_results = res
    nblk = sch.nblk
    outs = []
    for c in range(n_cores):
        zT = np.asarray(res.results[c]["zT"], np.float32)
        z = zT.reshape(P, nblk, OUTC).transpose(1, 0, 2).reshape(
            nblk * P, OUTC)[:sch.nslice]
        outs.append(z)
    return np.ascontiguousarray(np.concatenate(outs, axis=0))


if __name__ == "__main__":
    rng = np.random.default_rng(0)
    N, E = 4096, 60000
    src = rng.integers(0, N, E)
    dst = rng.integers(0, N, E)
    sch = build_schedule(src, dst, N, 8)
    print("ncols", sch.ncols, "gidx_cols", sch.gidx_cols, "maxL", sch.maxL)
    print("schedule numpy check rel err:",
          numpy_check_schedule(sch, src, dst, N))
    # full-size schedule stats
    src = rng.integers(0, 100000, 1600000)
    dst = rng.integers(0, 100000, 1600000)
    sch = build_schedule(src, dst, 100000, 8)
    tot_slots = sum(sum(Ls) for Ls in sch.gather_L)
    gathers = sum((L + GMAX - 1) // GMAX
                  for Ls in sch.gather_L for L in Ls)
    print(f"full: ncols={sch.ncols} slots/layer={tot_slots} "
          f"gathers/layer={gathers} maxL={sch.maxL} gidx_cols={sch.gidx_cols}")
